# revision 1
# baseline (speedup 1.0000x reference)
"""GCN block (GCNConv + BN(eval) + ReLU) on 8 Trainium2 NeuronCores.

Strategy (fully data-parallel, no collectives):
  out = relu(BN(D^{-1/2}(A+I)D^{-1/2} (x W) + b))
      = relu(dis_dst * ((sum_{e->dst} xs[src] + xs[dst]) @ W') + b')
  where xs = x * dis (dis = deg^{-1/2}), W' = W * s, b' = b*s + t (BN folded).

  Nodes are sharded across 8 cores by destination block.  Each core gathers
  the (host-compacted) source rows for its edges with dma_gather (bf16),
  reduces edge slots into 128-dst tiles with one-hot selection matmuls on the
  tensor engine (output transposed: [feat, dst], accumulated in PSUM f32),
  then applies the 512x512 transform GEMM, a K=1 bias matmul (bias pre-scaled
  by 1/dis so the final per-partition dis scale can be fused into the ReLU
  activation), and writes f32 output.
"""

import sys

if "/opt/trn_rl_repo" not in sys.path:
    sys.path.insert(0, "/opt/trn_rl_repo")

import math

import ml_dtypes
import numpy as np

BF16 = ml_dtypes.bfloat16

N_CORES = 8
P = 128
BN_EPS = 1e-5
ST_TILES = 16  # tiles per supertile (gather-source compaction granularity)


def _prep(x, edge_index, W, b, gamma, beta, running_mean, running_var):
    """Host-side preprocessing: sharding, edge sorting/compaction, BN folding.

    Returns (meta, in_maps): compile-time structure (uniform across cores)
    and per-core input tensors.
    """
    N, F = x.shape
    F_OUT = W.shape[1]
    KC = F // P
    assert N % N_CORES == 0
    NB = N // N_CORES
    T = math.ceil(NB / P)  # dst tiles per core

    src = np.asarray(edge_index[0], dtype=np.int64)
    dst = np.asarray(edge_index[1], dtype=np.int64)

    deg = 1.0 + np.bincount(dst, minlength=N).astype(np.float64)
    dis = (1.0 / np.sqrt(deg)).astype(np.float32)

    xs = (np.asarray(x, np.float32) * dis[:, None]).astype(BF16)

    # BN folding
    s = (np.asarray(gamma, np.float32)
         / np.sqrt(np.asarray(running_var, np.float32) + BN_EPS))
    t = np.asarray(beta, np.float32) - np.asarray(running_mean, np.float32) * s
    Wp = (np.asarray(W, np.float32) * s[None, :]).astype(BF16)
    bp = (np.asarray(b, np.float32) * s + t).astype(np.float32)
    wp = np.ascontiguousarray(Wp.reshape(KC, P, F_OUT).transpose(1, 0, 2))

    # ---- degree-balanced node -> (core, tile, slot) assignment (snake deal)
    NBINS = N_CORES * T
    order = np.argsort(-(deg - 1.0), kind="stable")
    assign = np.empty(N, np.int64)   # node -> bin
    slot_of = np.empty(N, np.int64)  # node -> slot within bin
    pos = 0
    rnd = 0
    while pos < N:
        chunk = order[pos:pos + NBINS]
        if rnd % 2 == 0:
            bins = np.arange(len(chunk))
        else:
            bins = NBINS - 1 - np.arange(len(chunk))
        assign[chunk] = bins
        slot_of[chunk] = rnd
        pos += NBINS
        rnd += 1
    assert rnd <= P, f"too many slot rounds {rnd}"
    core_of_bin = assign % N_CORES
    tile_of_bin = assign // N_CORES

    # node_map[k][t, p] = original node id (or -1)
    node_map = np.full((N_CORES, T, P), -1, dtype=np.int64)
    node_map[core_of_bin, tile_of_bin, slot_of] = np.arange(N)

    e_core = core_of_bin[dst]
    e_tile = tile_of_bin[dst]
    e_slot = slot_of[dst]

    n_st = math.ceil(T / ST_TILES)
    st_tile_lo = [st * ST_TILES for st in range(n_st)]
    st_tile_hi = [min((st + 1) * ST_TILES, T) for st in range(n_st)]

    # ---- pass 1: per-core edge lists sorted by tile, per-tile counts, uniqs
    per_core = []
    cnt = np.zeros((N_CORES, T), dtype=np.int64)
    uniq_cnt = np.zeros((N_CORES, n_st), dtype=np.int64)
    for k in range(N_CORES):
        m = e_core == k
        s_k = src[m]
        t_k = e_tile[m]
        p_k = e_slot[m]
        o = np.argsort(t_k, kind="stable")
        s_k, t_k, p_k = s_k[o], t_k[o], p_k[o]
        bounds = np.searchsorted(t_k, np.arange(T + 1))
        cnt[k] = bounds[1:] - bounds[:-1]
        st_data = []
        for st in range(n_st):
            e_lo, e_hi = bounds[st_tile_lo[st]], bounds[st_tile_hi[st]]
            u, inv = np.unique(s_k[e_lo:e_hi], return_inverse=True)
            uniq_cnt[k, st] = len(u)
            st_data.append((u, inv, e_lo, e_hi))
        per_core.append((s_k, t_k, p_k, bounds, st_data))

    S_t = (np.ceil(cnt.max(axis=0) / P).astype(np.int64) * P)
    S_t = np.maximum(S_t, P)
    off_t = np.concatenate([[0], np.cumsum(S_t)])
    TOT = int(off_t[-1])
    R_st = uniq_cnt.max(axis=0)
    base_st = np.concatenate([[0], np.cumsum(R_st)])
    GR = int(base_st[-1])
    assert R_st.max() <= 32767, f"supertile unique rows {R_st.max()} > int16"
    NG_t = (S_t // P).astype(np.int64)

    # ---- pass 2: per-core arrays
    in_maps = []
    for k in range(N_CORES):
        s_k, t_k, p_k, bounds, st_data = per_core[k]
        gsrc = np.zeros((GR, F), dtype=BF16)
        idx_flat = np.zeros(TOT, dtype=np.int16)
        dstl_flat = np.full(TOT, -1.0, dtype=np.float32)
        for st in range(n_st):
            u, inv, e_lo, e_hi = st_data[st]
            gsrc[base_st[st]:base_st[st] + len(u)] = xs[u]
            for tt in range(st_tile_lo[st], st_tile_hi[st]):
                t_lo, t_hi = bounds[tt], bounds[tt + 1]
                n_e = t_hi - t_lo
                o = off_t[tt]
                iv = inv[t_lo - e_lo:t_hi - e_lo]
                dv = p_k[t_lo:t_hi]
                so = np.argsort(iv, kind="stable")  # ascending rows => locality
                idx_flat[o:o + n_e] = iv[so].astype(np.int16)
                dstl_flat[o:o + n_e] = dv[so].astype(np.float32)
        gidx = np.zeros((P, TOT // 16), dtype=np.int16)
        wrapped = idx_flat.reshape(TOT // 16, 16).T
        for c in range(8):
            gidx[16 * c:16 * (c + 1), :] = wrapped
        # even tiles: host one-hot sel (DMA); odd tiles: dstl for DVE build
        sel_parts = []
        dstl_parts = []
        for tt in range(T):
            seg = slice(off_t[tt], off_t[tt + 1])
            if tt % 2 == 0 or True:
                oh = (dstl_flat[seg][:, None]
                      == np.arange(P, dtype=np.float32)[None, :])
                sel_parts.append(
                    oh.reshape(-1, P, P).transpose(1, 0, 2).reshape(P, -1))
            else:
                dstl_parts.append(
                    dstl_flat[seg].reshape(-1, P).T)  # [128, ng]
        sel_pk = np.ascontiguousarray(
            np.concatenate(sel_parts, axis=1)).astype(BF16)
        dstl_pk = (np.ascontiguousarray(np.concatenate(dstl_parts, axis=1))
                   .astype(np.float32) if dstl_parts
                   else np.zeros((P, 1), np.float32))
        iota = np.ascontiguousarray(np.broadcast_to(
            np.arange(P, dtype=np.float32), (P, P)).astype(BF16))

        nm = node_map[k]  # [T, P]
        valid = nm >= 0
        nm_safe = np.where(valid, nm, 0)
        dis_tp = np.where(valid, dis[nm_safe], 1.0).astype(np.float32)  # [T, P]
        dis_t = np.ascontiguousarray(dis_tp.T)  # [128, T]
        invdis = np.zeros((1, T * P), dtype=BF16)
        invdis[0, :] = np.where(valid, 1.0 / np.maximum(dis_tp, 1e-9), 0.0
                                ).reshape(-1).astype(BF16)
        xso_rows = np.where(valid[:, :, None], xs[nm_safe].astype(np.float32), 0.0)
        xs_own = np.ascontiguousarray(
            xso_rows.transpose(1, 0, 2)).astype(BF16)  # [128, T, F]
        ident = np.eye(P, dtype=np.float32).astype(BF16)
        in_maps.append({
            "xs_own": xs_own,
            "ident": np.ascontiguousarray(ident),
            "iota": iota,
            "gsrc": gsrc,
            "gidx": gidx,
            "sel_pk": sel_pk,
            "dstl_pk": dstl_pk,
            "dis_t": dis_t,
            "invdis": invdis,
            "wp": wp,
            "bp": bp.reshape(1, F_OUT).astype(BF16),
        })

    sel_off = []
    dve_off = []
    so = do = 0
    for tt in range(T):
        sel_off.append(so)
        dve_off.append(-1)
        so += int(S_t[tt])
    SEL_TOT, DVE_TOT = so, max(do, 1)

    meta = {
        "SEL_TOT": SEL_TOT, "DVE_TOT": DVE_TOT,
        "sel_off": sel_off, "dve_off": dve_off,
        "N": N, "F": F, "F_OUT": F_OUT, "KC": KC, "NB": NB, "T": T,
        "TOT": TOT, "GR": GR, "n_st": n_st,
        "S_t": S_t.tolist(), "off_t": off_t.tolist(), "NG_t": NG_t.tolist(),
        "R_st": [int(v) for v in R_st], "base_st": [int(v) for v in base_st],
        "st_tile_lo": st_tile_lo, "st_tile_hi": st_tile_hi,
        "node_map": node_map,
    }
    return meta, in_maps


def _build_program(meta):
    """Emit the Bass/Tile program (shared by all cores)."""
    import concourse.bacc as bacc
    import concourse.mybir as mybir
    import concourse.tile as tile

    F, F_OUT, KC = meta["F"], meta["F_OUT"], meta["KC"]
    NB, T, TOT, GR = meta["NB"], meta["T"], meta["TOT"], meta["GR"]
    S_t, off_t, NG_t = meta["S_t"], meta["off_t"], meta["NG_t"]
    base_st, R_st = meta["base_st"], meta["R_st"]
    n_st = meta["n_st"]
    st_of_tile = [st for st in range(n_st)
                  for _ in range(meta["st_tile_lo"][st], meta["st_tile_hi"][st])]

    dt = mybir.dt
    nc = bacc.Bacc("TRN2", target_bir_lowering=False, debug=False,
                   enable_asserts=False, num_devices=N_CORES,
                   num_swdge_queues=4)

    gsrc = nc.dram_tensor("gsrc", [GR, F], dt.bfloat16, kind="ExternalInput").ap()
    gidx = nc.dram_tensor("gidx", [P, TOT // 16], dt.int16, kind="ExternalInput").ap()
    sel_pk = nc.dram_tensor("sel_pk", [P, meta["SEL_TOT"]], dt.bfloat16, kind="ExternalInput").ap()
    dstl_pk = nc.dram_tensor("dstl_pk", [P, meta["DVE_TOT"]], dt.float32, kind="ExternalInput").ap()
    iota = nc.dram_tensor("iota", [P, P], dt.bfloat16, kind="ExternalInput").ap()
    dis_t = nc.dram_tensor("dis_t", [P, T], dt.float32, kind="ExternalInput").ap()
    invdis = nc.dram_tensor("invdis", [1, T * P], dt.bfloat16, kind="ExternalInput").ap()
    ident = nc.dram_tensor("ident", [P, P], dt.bfloat16, kind="ExternalInput").ap()
    xs_own = nc.dram_tensor("xs_own", [P, T, F], dt.bfloat16, kind="ExternalInput").ap()
    wp = nc.dram_tensor("wp", [P, KC, F_OUT], dt.bfloat16, kind="ExternalInput").ap()
    bp = nc.dram_tensor("bp", [1, F_OUT], dt.bfloat16, kind="ExternalInput").ap()
    out = nc.dram_tensor("out", [P, T, F_OUT], dt.float32, kind="ExternalOutput").ap()

    max_ng = max(NG_t)
    TB = 8
    max_bw = max(off_t[min(t0 + TB, T)] - off_t[t0] for t0 in range(0, T, TB))

    with tile.TileContext(nc) as tc:
        with (
            tc.tile_pool(name="const", bufs=1) as cpool,
            tc.tile_pool(name="gbuf", bufs=4) as gpool,
            tc.tile_pool(name="small", bufs=3) as spool,
            tc.tile_pool(name="sel", bufs=2) as selpool,
            tc.tile_pool(name="dve", bufs=24) as dvepool,
            tc.tile_pool(name="aggT", bufs=2) as aggpool,
            tc.tile_pool(name="outsb", bufs=3) as opool,
            tc.tile_pool(name="psA", bufs=2, space="PSUM") as psA,
            tc.tile_pool(name="psB", bufs=2, space="PSUM") as psB,
        ):
            # resident constants
            ident_sb = cpool.tile([P, P], dt.bfloat16, tag="ident")
            nc.sync.dma_start(ident_sb[:], ident[:])
            iota_sb = cpool.tile([P, P], dt.bfloat16, tag="iota")
            nc.sync.dma_start(iota_sb[:], iota[:])
            dis_sb = cpool.tile([P, T], dt.float32, tag="dis")
            nc.sync.dma_start(dis_sb[:], dis_t[:])
            invdis_sb = cpool.tile([1, T * P], dt.bfloat16, tag="invdis")
            nc.sync.dma_start(invdis_sb[:], invdis[:])
            wp_sb = cpool.tile([P, KC, F_OUT], dt.bfloat16, tag="wp")
            nc.sync.dma_start(wp_sb[:], wp[:])
            bp_sb = cpool.tile([1, F_OUT], dt.bfloat16, tag="bp")
            nc.sync.dma_start(bp_sb[:], bp[:])

            qn = [0]
            sel_off, dve_off = meta["sel_off"], meta["dve_off"]
            for t0 in range(0, T, TB):
                t1 = min(t0 + TB, T)
                nb_t = t1 - t0
                o16a, o16b = off_t[t0] // 16, off_t[t1] // 16
                offa, offb = off_t[t0], off_t[t1]

                idx_sb = spool.tile([P, max_bw // 16], dt.int16, tag="idx")
                nc.sync.dma_start(idx_sb[:, :o16b - o16a], gidx[:, o16a:o16b])
                # even tiles of the batch: one contiguous sel slice
                ev = [t for t in range(t0, t1)]
                od = []
                sla, slb = sel_off[ev[0]], sel_off[ev[-1]] + S_t[ev[-1]]
                sel_sb = selpool.tile([P, max_bw], dt.bfloat16, tag="sel")
                nc.sync.dma_start(sel_sb[:, :slb - sla], sel_pk[:, sla:slb])
                if od:
                    dva = dve_off[od[0]]
                    dvb = dve_off[od[-1]] + NG_t[od[-1]]
                    dstl_sb = spool.tile([P, max_bw // P], dt.float32, tag="dstl")
                    nc.sync.dma_start(dstl_sb[:, :dvb - dva], dstl_pk[:, dva:dvb])
                xso_sb = gpool.tile([P, TB, F], dt.bfloat16, tag="xso")
                nc.sync.dma_start(xso_sb[:, :nb_t, :], xs_own[:, t0:t1, :])
                out_blk = opool.tile([P, TB, F_OUT], dt.float32, tag="out_sb")

                for t in range(t0, t1):
                    st = st_of_tile[t]
                    ng = NG_t[t]
                    s_t = S_t[t]
                    loff = off_t[t] - offa  # slot offset within batch

                    g_sb = gpool.tile([P, max_ng, F], dt.bfloat16, tag="g")
                    # dma_gather caps at 1024 idxs (64 descs x 16 engines)
                    for g0 in range(0, ng, 8):
                        g1 = min(g0 + 8, ng)
                        cnt = (g1 - g0) * P
                        nc.gpsimd.dma_gather(
                            out_ap=g_sb[:, g0:g1, :],
                            in_ap=gsrc[base_st[st]:base_st[st] + R_st[st], :],
                            idxs_ap=idx_sb[:, (loff + g0 * P) // 16:
                                           (loff + g0 * P) // 16 + cnt // 16],
                            num_idxs=cnt,
                            num_idxs_reg=cnt,
                            elem_size=F,
                            queue_num=qn[0] % 4,
                        )
                        qn[0] += 1

                    # self-loop term: aggT[fchunk, dst] = xs_own_tile^T (rhs=I)
                    aggT_ps = psA.tile([P, F], dt.float32, tag="aggT_ps")
                    for c in range(KC):
                        nc.tensor.matmul(
                            aggT_ps[:, c * P:(c + 1) * P],
                            lhsT=xso_sb[:, t - t0, c * P:(c + 1) * P],
                            rhs=ident_sb[:],
                            start=(c == 0),
                            stop=False,
                            skip_group_check=True,
                        )
                    # selection matmuls: aggT[fchunk, dst] += G_chunk^T @ selR
                    if True:
                        s0 = sel_off[t] - sla
                        sels = [sel_sb[:, s0 + g * P:s0 + (g + 1) * P]
                                for g in range(ng)]
                    else:
                        d0 = dve_off[t] - dva
                        sels = []
                        for g in range(ng):
                            sd = dvepool.tile([P, P], dt.bfloat16, tag="seld")
                            nc.vector.tensor_scalar(
                                out=sd[:], in0=iota_sb[:],
                                scalar1=dstl_sb[:, d0 + g:d0 + g + 1],
                                scalar2=None,
                                op0=mybir.AluOpType.is_equal)
                            sels.append(sd[:])
                    for g in range(ng):
                        for c in range(KC):
                            nc.tensor.matmul(
                                aggT_ps[:, c * P:(c + 1) * P],
                                lhsT=g_sb[:, g, c * P:(c + 1) * P],
                                rhs=sels[g],
                                start=False,
                                stop=(g == ng - 1 and c == KC - 1),
                                skip_group_check=True,
                            )

                    aggT_sb = aggpool.tile([P, F], dt.bfloat16, tag="aggT_sb")
                    nc.vector.tensor_copy(aggT_sb[:], aggT_ps[:])

                    # transform GEMM + K=1 bias row (bias pre-scaled by 1/dis)
                    out_ps = psB.tile([P, F_OUT], dt.float32, tag="out_ps")
                    for c in range(KC):
                        nc.tensor.matmul(
                            out_ps[:],
                            lhsT=aggT_sb[:, c * P:(c + 1) * P],
                            rhs=wp_sb[:, c, :],
                            start=(c == 0),
                            stop=False,
                        )
                    nc.tensor.matmul(
                        out_ps[:],
                        lhsT=invdis_sb[:1, t * P:(t + 1) * P],
                        rhs=bp_sb[:1, :],
                        start=False,
                        stop=True,
                    )

                    nc.scalar.activation(
                        out_blk[:, t - t0, :],
                        out_ps[:],
                        mybir.ActivationFunctionType.Relu,
                        scale=dis_sb[:, t:t + 1],
                    )

                nc.sync.dma_start(out[:, t0:t1, :], out_blk[:, :nb_t, :])

    nc.compile()
    return nc


_CACHE = {}


def _get_program(meta):
    key = (meta["N"], meta["F"], meta["F_OUT"], meta["TOT"], meta["GR"],
           tuple(meta["S_t"]), tuple(meta["R_st"]))
    if key not in _CACHE:
        _CACHE[key] = _build_program(meta)
    return _CACHE[key]


def kernel(x, edge_index, W, b, gamma, beta, running_mean, running_var,
           _want_results_holder=None, _run_kwargs=None):
    meta, in_maps = _prep(x, edge_index, W, b, gamma, beta,
                          running_mean, running_var)
    nc = _get_program(meta)

    from concourse.bass_utils import run_bass_kernel_spmd

    res = run_bass_kernel_spmd(nc, in_maps, core_ids=list(range(N_CORES)),
                               **(_run_kwargs or {}))
    if _want_results_holder is not None:
        _want_results_holder.append((nc, meta, in_maps, res))

    T, F_OUT = meta["T"], meta["F_OUT"]
    node_map = meta["node_map"]
    out = np.empty((meta["N"], F_OUT), dtype=np.float32)
    for k in range(N_CORES):
        tiled = res.results[k]["out"]  # [128, T, F_OUT]
        rows = np.ascontiguousarray(tiled.transpose(1, 0, 2))  # [T, 128, F]
        nm = node_map[k]
        valid = nm >= 0
        out[nm[valid]] = rows[valid]
    return out

